# revision 1
# baseline (speedup 1.0000x reference)
"""Trainium2 Bass kernel for nn_Block_46566035423950 (dense transformer block).

Full inputs in, full outputs out. Data-parallel over batch across 8 NeuronCores
(256 batch elements per core). Within a core, batch is processed in chunks of
8 batch elements (one "octet" = 120 tokens), with all GEMMs running
feature-major on the tensor engine in bf16 (fp32 accumulation in PSUM).

Layout strategy (per core):
  - joint/relation loaded token-major [120, 1024] fp32; LayerNorm stats via
    bn_stats; normalize+cast to bf16 on ACT; transpose to feature-major
    [128, 8, 128] via the HW xbar DMA transpose (bf16).
  - QKV GEMMs feature-major with head-pair-permuted weight columns so that
    per-head J/I "stacks" (contraction dim 128 = [J 64 | I 64]) can be built
    with lane-aligned PSUM->SBUF copies.
  - Attention per (chunk, head): one K=128 matmul gives all 8 batch elements'
    score blocks (cross-batch garbage masked later); attn_lin accumulates into
    the same PSUM via a replicated W_Iconv stationary (K=64).
    exp on ACT (scale=1/8 folded in), block-diagonal mask multiply on DVE.
  - V is computed token-major directly (M=120 option-B GEMM), so P@V becomes
    lhsT=V-slice, rhs=E^T with no transposes. Row sums via a ones-column
    matmul; normalization folded into the PSUM->SBUF copy of x^T.
  - proj and fc2 run "option-B" (lhsT = activations' token columns) so their
    outputs come out token-major for the residual adds and the final store.
  - LayerNorm gammas/betas are folded into the weights host-side (W' = g*W,
    bias rows b@W added via ACT bias / const tiles), so the kernel is correct
    for arbitrary gamma/beta.
"""

import numpy as np
import ml_dtypes
from contextlib import ExitStack

B, N, C, H, HS = 2048, 15, 1024, 16, 64
NCORES = 8
BC = B // NCORES      # 256 batch per core
OB = 8                # batch per chunk (octet)
T = OB * N            # 120 tokens per chunk
PT = 128              # padded token width (dma-transpose tile)
SCALE = HS ** -0.5
EPS = 1e-5
MLP_ACT = "Gelu"  # test_sim overrides (CoreSim lacks Gelu)


def _build(tc, io, nchunk, dbg=None):
    import concourse.bass as bass
    import concourse.mybir as mybir

    dt = mybir.dt
    AF = mybir.ActivationFunctionType
    ALU = mybir.AluOpType
    nc = tc.nc
    ctx = tc.ctx  # ExitStack attached by caller

    jf, rf = io["joint_feature"], io["relation_feature"]
    out = io["out"]

    # ---------------- pools ----------------
    wpool = ctx.enter_context(tc.tile_pool(name="w", bufs=1))
    cpool = ctx.enter_context(tc.tile_pool(name="c", bufs=1))
    tokp = ctx.enter_context(tc.tile_pool(name="tok", bufs=2))
    tokp1 = ctx.enter_context(tc.tile_pool(name="tok1", bufs=1))
    fmp = ctx.enter_context(tc.tile_pool(name="fm", bufs=2))
    fmp1 = ctx.enter_context(tc.tile_pool(name="fm1", bufs=1))
    smp = ctx.enter_context(tc.tile_pool(name="sm", bufs=3))
    attn_sb = ctx.enter_context(tc.tile_pool(name="asb", bufs=3))
    gemm_ps = ctx.enter_context(tc.tile_pool(name="gps", bufs=2, space="PSUM"))
    attn_ps = ctx.enter_context(tc.tile_pool(name="aps", bufs=3, space="PSUM"))
    pv_ps = ctx.enter_context(tc.tile_pool(name="pps", bufs=2, space="PSUM"))

    # ---------------- weights (resident, bf16, permuted) ----------------
    # wj/wi: [128, 8(cin), 3072]; q sec cols 128t+[0:64]=head a(t), [64:128]=head b(t)
    #   J: (a,b) = (t, t+8) ; I: (a,b) = (t+8, t); v-sec: J pairs (t,t+8), I natural
    wj = wpool.tile([128, 8, 3072], dt.bfloat16)
    wi = wpool.tile([128, 8, 3072], dt.bfloat16)
    wp = wpool.tile([128, 8, 1024], dt.bfloat16)   # rows pair-permuted (t, t+8)
    wf1 = wpool.tile([128, 8, 512], dt.bfloat16)
    wf2 = wpool.tile([128, 4, 1024], dt.bfloat16)
    wconv2 = wpool.tile([128, 120], dt.bfloat16)   # W_Iconv replicated 8x along free

    def load_qkv(dst, src, jlike):
        # src dram [1024, 3072] fp32 (already g-scaled host-side)
        s = src
        for sec in range(2):  # q, k sections: pair-permuted
            for half in range(2):
                # half 0 -> slot [0:64]; half 1 -> slot [64:128] of each pair col
                if jlike:
                    h0 = 0 if half == 0 else 8
                else:
                    h0 = 8 if half == 0 else 0
                # dest cols sec*1024 + 128*t + 64*half + c ; heads h0+t
                d = dst[:, :, :].rearrange("p a (s t o c) -> p a s t o c", s=3, t=8, o=2, c=64)
                sr = s.rearrange("(a p) (s hh c) -> p a s hh c", p=128, s=3, c=64)
                for a in range(8):
                    nc.gpsimd.dma_start(out=d[:, a, sec, :, half, :],
                                        in_=sr[:, a, sec, h0:h0 + 8, :])
        # v section
        if jlike:
            for half in range(2):
                h0 = 0 if half == 0 else 8
                d = dst[:, :, :].rearrange("p a (s t o c) -> p a s t o c", s=3, t=8, o=2, c=64)
                sr = s.rearrange("(a p) (s hh c) -> p a s hh c", p=128, s=3, c=64)
                for a in range(8):
                    nc.gpsimd.dma_start(out=d[:, a, 2, :, half, :],
                                        in_=sr[:, a, 2, h0:h0 + 8, :])
        else:
            dsl = dst[:, :, 2048:3072]
            ssl = s.rearrange("(a p) n -> p a n", p=128)[:, :, 2048:3072]
            nc.gpsimd.dma_start(out=dsl, in_=ssl)

    load_qkv(wj, io["w_jqkv"], True)
    load_qkv(wi, io["w_iqk"], False)

    # W_proj rows pair-permuted: partitions 0:64 <- heads 0..7, 64:128 <- heads 8..15
    # wp[p, a, n]: p<64 -> W_proj[64*a + p, n]; p>=64 -> W_proj[512 + 64*a + (p-64), n]
    nc.gpsimd.dma_start(out=wp[0:64, :, :],
                        in_=io["w_proj"][0:512, :].rearrange("(a p) n -> p a n", p=64))
    nc.gpsimd.dma_start(out=wp[64:128, :, :],
                        in_=io["w_proj"][512:1024, :].rearrange("(a p) n -> p a n", p=64))

    nc.gpsimd.dma_start(out=wf1[:], in_=io["w_fc1"].rearrange("(a p) n -> p a n", p=128))
    nc.gpsimd.dma_start(out=wf2[:], in_=io["w_fc2"].rearrange("(a p) n -> p a n", p=128))
    for g in range(8):
        nc.gpsimd.dma_start(out=wconv2[0:64, 15 * g:15 * g + 15], in_=io["W_Iconv"])
        nc.gpsimd.dma_start(out=wconv2[64:128, 15 * g:15 * g + 15], in_=io["W_Iconv"])

    # ---------------- consts ----------------
    maskt = cpool.tile([120, 120], dt.bfloat16)
    nc.sync.dma_start(maskt[:], io["mask_c"])
    bj_c = cpool.tile([128, 16], dt.float32)
    nc.sync.dma_start(bj_c[:], io["bj_c"])
    bi_c = cpool.tile([128, 24], dt.float32)
    nc.sync.dma_start(bi_c[:], io["bi_c"])
    bvb = cpool.tile([120, 1024], dt.bfloat16)
    nc.gpsimd.dma_start(bvb[:], io["bvb_c"])
    bprojb = cpool.tile([120, 1024], dt.bfloat16)
    nc.gpsimd.dma_start(bprojb[:], io["bprojb_c"])
    bfc2b = cpool.tile([120, 1024], dt.bfloat16)
    nc.gpsimd.dma_start(bfc2b[:], io["bfc2b_c"])
    bf1t = cpool.tile([128, 4], dt.float32)
    nc.sync.dma_start(bf1t[:], io["bfc1t_c"])
    onesc = cpool.tile([120, 1], dt.bfloat16)
    nc.gpsimd.memset(onesc[:], 1.0)
    epst = cpool.tile([128, 1], dt.float32)
    nc.gpsimd.memset(epst[:], EPS)

    # persistent zero-padded LN output slots (rows 120..127 stay zero)
    nslots = []
    for i in range(4):
        t_ = cpool.tile([128, 1024], dt.bfloat16, tag=f"nslot{i}")
        nc.gpsimd.memset(t_[:], 0.0)
        nslots.append(t_)

    jf2 = jf.rearrange("b n c -> (b n) c")
    rf2 = rf.rearrange("b n c -> (b n) c")
    out2 = out.rearrange("b n c -> (b n) c")

    def layernorm_to(dst_slot, src_tile):
        st = smp.tile([120, 2, 6], dt.float32, tag="lnst")
        nc.vector.bn_stats(st[:, 0, :], src_tile[:, 0:512])
        nc.vector.bn_stats(st[:, 1, :], src_tile[:, 512:1024])
        mv = smp.tile([120, 2], dt.float32, tag="lnmv")
        nc.vector.bn_aggr(mv[:], st[:])
        sd = smp.tile([120, 1], dt.float32, tag="lnsd")
        nc.scalar.activation(sd[:], mv[:, 1:2], AF.Sqrt, bias=epst[0:120, :])
        rstd = smp.tile([120, 1], dt.float32, tag="lnrs")
        nc.vector.reciprocal(rstd[:], sd[:])
        nmr = smp.tile([120, 1], dt.float32, tag="lnnm")
        nc.vector.tensor_scalar(nmr[:], mv[:, 0:1], rstd[:], -1.0, ALU.mult, ALU.mult)
        nc.scalar.activation(dst_slot[0:120, :], src_tile[:], AF.Identity,
                             bias=nmr[:], scale=rstd[:])

    # ---------------- main chunk loop ----------------
    for ck in range(nchunk):
        r0 = ck * T

        jt = tokp.tile([120, 1024], dt.float32, tag="jt")
        nc.sync.dma_start(jt[:], jf2[r0:r0 + T, :])
        rt = tokp1.tile([120, 1024], dt.float32, tag="rt_ot")
        nc.sync.dma_start(rt[:], rf2[r0:r0 + T, :])

        xjn = nslots[(ck % 2) * 2 + 0]
        xin = nslots[(ck % 2) * 2 + 1]
        layernorm_to(xjn, jt)
        layernorm_to(xin, rt)

        xjT = fmp.tile([128, 8, 128], dt.bfloat16, tag="xjT")
        nc.sync.dma_start(xjT[:], xjn[:], transpose=True)
        xiT = fmp.tile([128, 8, 128], dt.bfloat16, tag="xiT")
        nc.sync.dma_start(xiT[:], xin[:], transpose=True)

        # ---- QKV GEMMs -> stacks ----
        Qs = fmp1.tile([128, 16, 120], dt.bfloat16, tag="Qs")
        Ks = fmp1.tile([128, 16, 120], dt.bfloat16, tag="Ks")
        IvT = fmp1.tile([128, 8, 128], dt.bfloat16, tag="IvT")

        for sec in range(2):           # 0=q, 1=k
            dst = Qs if sec == 0 else Ks
            for t in range(8):
                ps = gemm_ps.tile([128, 128], dt.float32, tag="gps")
                for c in range(8):
                    nc.tensor.matmul(ps[:], wj[:, c, sec * 1024 + 128 * t:sec * 1024 + 128 * t + 128],
                                     xjT[:, c, :], start=(c == 0), stop=(c == 7))
                # J pairs (t, t+8): top -> stack t rows 0:64, bottom -> stack t+8 rows 64:128
                nc.scalar.activation(dst[0:64, t, :], ps[0:64, 0:120], AF.Identity,
                                     bias=bj_c[0:64, sec * 8 + t:sec * 8 + t + 1])
                nc.scalar.activation(dst[64:128, t + 8, :], ps[64:128, 0:120], AF.Identity,
                                     bias=bj_c[64:128, sec * 8 + t:sec * 8 + t + 1])
            for t in range(8):
                ps = gemm_ps.tile([128, 128], dt.float32, tag="gps")
                for c in range(8):
                    nc.tensor.matmul(ps[:], wi[:, c, sec * 1024 + 128 * t:sec * 1024 + 128 * t + 128],
                                     xiT[:, c, :], start=(c == 0), stop=(c == 7))
                # I pairs (t+8, t): top -> stack t+8 rows 0:64, bottom -> stack t rows 64:128
                nc.scalar.activation(dst[0:64, t + 8, :], ps[0:64, 0:120], AF.Identity,
                                     bias=bi_c[0:64, sec * 8 + t:sec * 8 + t + 1])
                nc.scalar.activation(dst[64:128, t, :], ps[64:128, 0:120], AF.Identity,
                                     bias=bi_c[64:128, sec * 8 + t:sec * 8 + t + 1])

        for t in range(8):             # Iv feature-major (natural head pairs)
            ps = gemm_ps.tile([128, 128], dt.float32, tag="gps")
            for c in range(8):
                nc.tensor.matmul(ps[:], wi[:, c, 2048 + 128 * t:2048 + 128 * t + 128],
                                 xiT[:, c, :], start=(c == 0), stop=(c == 7))
            nc.scalar.activation(IvT[:, t, :], ps[:], AF.Identity,
                                 bias=bi_c[:, 16 + t:16 + t + 1])

        # ---- V token-major (option-B), pair-permuted head cols ----
        vtok = fmp1.tile([128, 1024], dt.bfloat16, tag="vtok")
        for half in range(2):
            ps = gemm_ps.tile([120, 512], dt.float32, tag="gps")
            for c in range(8):
                nc.tensor.matmul(ps[:], xjT[:, c, 0:120],
                                 wj[:, c, 2048 + 512 * half:2048 + 512 * half + 512],
                                 start=(c == 0), stop=(c == 7))
            nc.vector.scalar_tensor_tensor(
                vtok[0:120, 512 * half:512 * half + 512], ps[:], 0.0,
                bvb[:, 512 * half:512 * half + 512], ALU.bypass, ALU.add)

        # ---- attention ----
        if dbg is not None and ck == 0:
            dbg_rs_sb = cpool.tile([1, 120], dt.float32, tag="dbgrs")
            dbg_pv_sb = cpool.tile([128, 120], dt.float32, tag="dbgpv")
        xTx = fmp1.tile([128, 8, 128], dt.bfloat16, tag="xTx")
        for hp in range(8):
            ems = []
            rrs = []
            rcat = smp.tile([1, 240], dt.float32, tag="rcat")
            for hi, h in enumerate((hp, hp + 8)):
                sc = attn_ps.tile([120, 120], dt.float32, tag="aps")
                nc.tensor.matmul(sc[:], Ks[:, h, :], Qs[:, h, :],
                                 start=True, stop=False)
                base = (h % 2) * 64
                nc.tensor.matmul(sc[:], wconv2[base:base + 64, :],
                                 IvT[base:base + 64, h // 2, 0:120],
                                 start=False, stop=True)
                ef = attn_sb.tile([120, 120], dt.bfloat16, tag="ef")
                nc.scalar.activation(ef[:], sc[:], AF.Exp, scale=SCALE)
                em = attn_sb.tile([120, 120], dt.bfloat16, tag="em")
                nc.vector.tensor_mul(em[:], ef[:], maskt[:])
                if dbg is not None and ck == 0 and h == 0:
                    dbg_em0 = em
                rs = attn_ps.tile([1, 120], dt.float32, tag="aps")
                nc.tensor.matmul(rs[:], onesc[:], em[:], start=True, stop=True)
                nc.vector.reciprocal(rcat[:, 120 * hi:120 * hi + 120], rs[:])
                if dbg is not None and ck == 0 and h == 0:
                    nc.vector.tensor_copy(dbg_rs_sb[:], rs[:])
                ems.append(em)
            pv = pv_ps.tile([128, 120], dt.float32, tag="pps")
            nc.tensor.matmul(pv[0:64, :], vtok[0:120, 128 * hp:128 * hp + 64],
                             ems[0][:], start=True, stop=True, tile_position=(0, 0))
            nc.tensor.matmul(pv[64:128, :], vtok[0:120, 128 * hp + 64:128 * hp + 128],
                             ems[1][:], start=True, stop=True, tile_position=(0, 64))
            rb = smp.tile([128, 240], dt.float32, tag="rb")
            nc.gpsimd.partition_broadcast(rb[:], rcat[:])
            nc.vector.tensor_mul(xTx[0:64, hp, 0:120], pv[0:64, :], rb[0:64, 0:120])
            nc.vector.tensor_mul(xTx[64:128, hp, 0:120], pv[64:128, :], rb[64:128, 120:240])
            if dbg is not None and ck == 0 and hp == 0:
                nc.sync.dma_start(dbg["rb0"], rb[:, 0:120])
                nc.vector.tensor_copy(dbg_pv_sb[:], pv[:])

        # ---- proj (option-B) + residual ----
        nc.vector.tensor_add(jt[:], jt[:], bprojb[:])
        jt2 = tokp.tile([120, 1024], dt.float32, tag="jt2")
        for half in range(2):
            ps = gemm_ps.tile([120, 512], dt.float32, tag="gps")
            for c in range(8):
                nc.tensor.matmul(ps[:], xTx[:, c, 0:120],
                                 wp[:, c, 512 * half:512 * half + 512],
                                 start=(c == 0), stop=(c == 7))
            nc.vector.scalar_tensor_tensor(
                jt2[:, 512 * half:512 * half + 512], ps[:], 0.0,
                jt[:, 512 * half:512 * half + 512], ALU.bypass, ALU.add)

        # ---- MLP ----
        xm = nslots[(ck % 2) * 2 + 0]
        layernorm_to(xm, jt2)
        xmT = fmp1.tile([128, 8, 128], dt.bfloat16, tag="xmT")
        nc.sync.dma_start(xmT[:], xm[:], transpose=True)

        h1T = fmp1.tile([128, 4, 128], dt.bfloat16, tag="h1T")
        for t in range(4):
            ps = gemm_ps.tile([128, 128], dt.float32, tag="gps")
            for c in range(8):
                nc.tensor.matmul(ps[:], wf1[:, c, 128 * t:128 * t + 128],
                                 xmT[:, c, :], start=(c == 0), stop=(c == 7))
            nc.scalar.activation(h1T[:, t, :], ps[:], getattr(AF, MLP_ACT),
                                 bias=bf1t[:, t:t + 1])

        nc.vector.tensor_add(jt2[:], jt2[:], bfc2b[:])
        ot = tokp1.tile([120, 1024], dt.float32, tag="rt_ot")
        for half in range(2):
            ps = gemm_ps.tile([120, 512], dt.float32, tag="gps")
            for c in range(4):
                nc.tensor.matmul(ps[:], h1T[:, c, 0:120],
                                 wf2[:, c, 512 * half:512 * half + 512],
                                 start=(c == 0), stop=(c == 3))
            nc.vector.scalar_tensor_tensor(
                ot[:, 512 * half:512 * half + 512], ps[:], 0.0,
                jt2[:, 512 * half:512 * half + 512], ALU.bypass, ALU.add)

        nc.sync.dma_start(out2[r0:r0 + T, :], ot[:])
        if dbg is not None and ck == 0:
            for nm, tl in [("xjn", xjn), ("xin", xin)]:
                nc.sync.dma_start(dbg[nm], tl[:])
            nc.sync.dma_start(dbg["xjT"], xjT[:])
            nc.sync.dma_start(dbg["Qs"], Qs[:])
            nc.sync.dma_start(dbg["Ks"], Ks[:])
            nc.sync.dma_start(dbg["IvT"], IvT[:])
            nc.sync.dma_start(dbg["vtok"], vtok[:])
            nc.sync.dma_start(dbg["xTx"], xTx[:])
            nc.sync.dma_start(dbg["jt2"], jt2[:])
            nc.sync.dma_start(dbg["h1T"], h1T[:])
            nc.sync.dma_start(dbg["em0"], dbg_em0[:])
            nc.sync.dma_start(dbg["xm"], xm[:])
            nc.sync.dma_start(dbg["rs0"], dbg_rs_sb[:])
            nc.sync.dma_start(dbg["pv0"], dbg_pv_sb[:])


def make_consts(W_Jqkv, W_Iqk, W_proj, b_proj, g1, b1, g2, b2, g3, b3,
                W_fc1, b_fc1, W_fc2, b_fc2):
    """Host-side preprocessing: fold LN gamma into weights, compute bias rows."""
    bf16 = ml_dtypes.bfloat16
    wj = (g1[:, None] * W_Jqkv).astype(np.float32)
    wi = (g2[:, None] * W_Iqk).astype(np.float32)
    wf1 = (g3[:, None] * W_fc1).astype(np.float32)
    bjrow = (b1 @ W_Jqkv).astype(np.float32)      # [3072]
    birow = (b2 @ W_Iqk).astype(np.float32)       # [3072]
    bw1row = (b3 @ W_fc1).astype(np.float32)      # [512]

    # mask: block-diag ones [120,120]
    m = np.zeros((120, 120), np.float32)
    for i in range(8):
        m[15 * i:15 * i + 15, 15 * i:15 * i + 15] = 1.0
    mask_c = m.astype(bf16)

    # bj_c [128, 16]: J stacks, pairs (t, t+8): col sec*8+t
    bj = np.zeros((128, 16), np.float32)
    bi = np.zeros((128, 24), np.float32)
    for sec in range(2):
        for t in range(8):
            bj[0:64, sec * 8 + t] = bjrow[sec * 1024 + 64 * t: sec * 1024 + 64 * t + 64]
            bj[64:128, sec * 8 + t] = bjrow[sec * 1024 + 64 * (t + 8): sec * 1024 + 64 * (t + 8) + 64]
            bi[0:64, sec * 8 + t] = birow[sec * 1024 + 64 * (t + 8): sec * 1024 + 64 * (t + 8) + 64]
            bi[64:128, sec * 8 + t] = birow[sec * 1024 + 64 * t: sec * 1024 + 64 * t + 64]
    for t in range(8):
        bi[:, 16 + t] = birow[2048 + 128 * t: 2048 + 128 * t + 128]

    # V token-major bias: cols pair-permuted (t, t+8)
    bvrow = bjrow[2048:3072]
    bvperm = np.zeros(1024, np.float32)
    for t in range(8):
        bvperm[128 * t:128 * t + 64] = bvrow[64 * t:64 * t + 64]
        bvperm[128 * t + 64:128 * t + 128] = bvrow[64 * (t + 8):64 * (t + 8) + 64]
    bvb_c = np.tile(bvperm[None, :], (120, 1)).astype(np.float32)

    bprojb_c = np.tile(b_proj[None, :], (120, 1)).astype(np.float32)
    bfc2b_c = np.tile(b_fc2[None, :], (120, 1)).astype(np.float32)
    bfc1t_c = (b_fc1 + bw1row).reshape(4, 128).T.astype(np.float32).copy()
    return dict(w_jqkv=wj, w_iqk=wi, w_fc1=wf1, mask_c=mask_c, bj_c=bj, bi_c=bi,
                bvb_c=bvb_c, bprojb_c=bprojb_c, bfc2b_c=bfc2b_c, bfc1t_c=bfc1t_c)


def build_nc(nchunk=BC // OB, bc=BC, debug=False):
    import concourse.bacc as bacc
    import concourse.tile as tile
    import concourse.mybir as mybir
    from contextlib import ExitStack

    dt = mybir.dt
    nc = bacc.Bacc("TRN2", target_bir_lowering=False, debug=False,
                   num_devices=NCORES)
    io = {}
    io["joint_feature"] = nc.dram_tensor("joint_feature", [bc, N, C], dt.float32,
                                         kind="ExternalInput").ap()
    io["relation_feature"] = nc.dram_tensor("relation_feature", [bc, N, C], dt.float32,
                                            kind="ExternalInput").ap()
    io["w_jqkv"] = nc.dram_tensor("w_jqkv", [C, 3 * C], dt.float32, kind="ExternalInput").ap()
    io["w_iqk"] = nc.dram_tensor("w_iqk", [C, 3 * C], dt.float32, kind="ExternalInput").ap()
    io["W_Iconv"] = nc.dram_tensor("W_Iconv", [HS, 15], dt.float32, kind="ExternalInput").ap()
    io["w_proj"] = nc.dram_tensor("w_proj", [C, C], dt.float32, kind="ExternalInput").ap()
    io["w_fc1"] = nc.dram_tensor("w_fc1", [C, C // 2], dt.float32, kind="ExternalInput").ap()
    io["w_fc2"] = nc.dram_tensor("w_fc2", [C // 2, C], dt.float32, kind="ExternalInput").ap()
    io["mask_c"] = nc.dram_tensor("mask_c", [120, 120], dt.bfloat16, kind="ExternalInput").ap()
    io["bj_c"] = nc.dram_tensor("bj_c", [128, 16], dt.float32, kind="ExternalInput").ap()
    io["bi_c"] = nc.dram_tensor("bi_c", [128, 24], dt.float32, kind="ExternalInput").ap()
    io["bvb_c"] = nc.dram_tensor("bvb_c", [120, 1024], dt.float32, kind="ExternalInput").ap()
    io["bprojb_c"] = nc.dram_tensor("bprojb_c", [120, 1024], dt.float32, kind="ExternalInput").ap()
    io["bfc2b_c"] = nc.dram_tensor("bfc2b_c", [120, 1024], dt.float32, kind="ExternalInput").ap()
    io["bfc1t_c"] = nc.dram_tensor("bfc1t_c", [128, 4], dt.float32, kind="ExternalInput").ap()
    io["out"] = nc.dram_tensor("out", [bc, N, C], dt.float32, kind="ExternalOutput").ap()
    dbg = None
    if debug:
        dbg = {}
        for nm, shp, dt_ in [("xjn", [128, 1024], dt.bfloat16), ("xin", [128, 1024], dt.bfloat16),
                             ("xjT", [128, 8, 128], dt.bfloat16), ("Qs", [128, 16, 120], dt.bfloat16),
                             ("Ks", [128, 16, 120], dt.bfloat16), ("IvT", [128, 8, 128], dt.bfloat16),
                             ("vtok", [128, 1024], dt.bfloat16), ("xTx", [128, 8, 128], dt.bfloat16),
                             ("jt2", [120, 1024], dt.float32), ("h1T", [128, 4, 128], dt.bfloat16),
                             ("em0", [120, 120], dt.bfloat16), ("xm", [128, 1024], dt.bfloat16),
                             ("rs0", [1, 120], dt.float32),
                             ("rb0", [128, 120], dt.float32), ("pv0", [128, 120], dt.float32)]:
            dbg[nm] = nc.dram_tensor("dbg_" + nm, shp, dt_, kind="ExternalOutput").ap()

    with tile.TileContext(nc) as tc:
        with ExitStack() as ctx:
            tc.ctx = ctx
            _build(tc, io, nchunk, dbg=dbg)
    nc.compile()
    return nc


def kernel(joint_feature, relation_feature, W_Jqkv, W_Iqk, W_Iconv, W_proj, b_proj,
           g_attn1, b_attn1, g_attn2, b_attn2, g_joint, b_joint,
           W_fc1, b_fc1, W_fc2, b_fc2):
    from concourse.bass_utils import run_bass_kernel_spmd

    consts = make_consts(np.asarray(W_Jqkv), np.asarray(W_Iqk), np.asarray(W_proj),
                         np.asarray(b_proj), np.asarray(g_attn1), np.asarray(b_attn1),
                         np.asarray(g_attn2), np.asarray(b_attn2), np.asarray(g_joint),
                         np.asarray(b_joint), np.asarray(W_fc1), np.asarray(b_fc1),
                         np.asarray(W_fc2), np.asarray(b_fc2))
    nc = build_nc()
    jf = np.ascontiguousarray(np.asarray(joint_feature, dtype=np.float32))
    rf = np.ascontiguousarray(np.asarray(relation_feature, dtype=np.float32))
    shared = dict(consts)
    shared["W_Iconv"] = np.asarray(W_Iconv, dtype=np.float32)
    shared["w_proj"] = np.asarray(W_proj, dtype=np.float32)
    shared["w_fc2"] = np.asarray(W_fc2, dtype=np.float32)
    in_maps = []
    for c in range(NCORES):
        m = dict(shared)
        m["joint_feature"] = jf[c * BC:(c + 1) * BC]
        m["relation_feature"] = rf[c * BC:(c + 1) * BC]
        in_maps.append(m)
    res = run_bass_kernel_spmd(nc, in_maps, list(range(NCORES)))
    outs = [res.results[c]["out"] for c in range(NCORES)]
    return np.concatenate(outs, axis=0).astype(np.float32)



# revision 41
# speedup vs baseline: 1.0565x; 1.0565x over previous
"""Trainium2 Bass kernel for nn_Block_46566035423950 (dense transformer block).

Full inputs in, full outputs out. Data-parallel over batch across 8 NeuronCores
(256 batch elements per core). Within a core, batch is processed in chunks of
8 batch elements (one "octet" = 120 tokens), with all GEMMs running
feature-major on the tensor engine in bf16 (fp32 accumulation in PSUM).

Layout strategy (per core):
  - joint/relation loaded token-major [120, 1024] fp32; LayerNorm stats via
    bn_stats; normalize+cast to bf16 on ACT; transpose to feature-major
    [128, 8, 128] via the HW xbar DMA transpose (bf16).
  - QKV GEMMs feature-major with head-pair-permuted weight columns so that
    per-head J/I "stacks" (contraction dim 128 = [J 64 | I 64]) can be built
    with lane-aligned PSUM->SBUF copies.
  - Attention per (chunk, head): one K=128 matmul gives all 8 batch elements'
    score blocks (cross-batch garbage masked later); attn_lin accumulates into
    the same PSUM via a replicated W_Iconv stationary (K=64).
    exp on ACT (scale=1/8 folded in), block-diagonal mask multiply on DVE.
  - V is computed token-major directly (M=120 option-B GEMM), so P@V becomes
    lhsT=V-slice, rhs=E^T with no transposes. Row sums via a ones-column
    matmul; normalization folded into the PSUM->SBUF copy of x^T.
  - proj and fc2 run "option-B" (lhsT = activations' token columns) so their
    outputs come out token-major for the residual adds and the final store.
  - LayerNorm gammas/betas are folded into the weights host-side (W' = g*W,
    bias rows b@W added via ACT bias / const tiles), so the kernel is correct
    for arbitrary gamma/beta.
"""

import numpy as np
import ml_dtypes
from contextlib import ExitStack

B, N, C, H, HS = 2048, 15, 1024, 16, 64
NCORES = 8
BC = B // NCORES      # 256 batch per core
OB = 8                # batch per chunk (octet)
T = OB * N            # 120 tokens per chunk
PT = 128              # padded token width (dma-transpose tile)
SCALE = HS ** -0.5
EPS = 1e-5
MLP_ACT = "Gelu"  # test_sim overrides (CoreSim lacks Gelu)


def _build(tc, io, nchunk, dbg=None):
    import concourse.bass as bass
    import concourse.mybir as mybir

    dt = mybir.dt
    AF = mybir.ActivationFunctionType
    ALU = mybir.AluOpType
    nc = tc.nc
    ctx = tc.ctx  # ExitStack attached by caller

    jf, rf = io["joint_feature"], io["relation_feature"]
    out = io["out"]

    # ---------------- pools ----------------
    wpool = ctx.enter_context(tc.tile_pool(name="w", bufs=1))
    cpool = ctx.enter_context(tc.tile_pool(name="c", bufs=1))
    tokp = ctx.enter_context(tc.tile_pool(name="tok", bufs=2))
    tokp1 = ctx.enter_context(tc.tile_pool(name="tok1", bufs=1))
    fmp = ctx.enter_context(tc.tile_pool(name="fm", bufs=2))
    fmp1 = ctx.enter_context(tc.tile_pool(name="fm1", bufs=1))
    smp = ctx.enter_context(tc.tile_pool(name="sm", bufs=3))
    attn_sb = ctx.enter_context(tc.tile_pool(name="asb", bufs=3))
    gemm_ps = ctx.enter_context(tc.tile_pool(name="gps", bufs=2, space="PSUM"))
    attn_ps = ctx.enter_context(tc.tile_pool(name="aps", bufs=3, space="PSUM"))
    pv_ps = ctx.enter_context(tc.tile_pool(name="pps", bufs=2, space="PSUM"))

    # ---------------- weights (resident, bf16, permuted) ----------------
    # wj/wi: [128, 8(cin), 3072]; q sec cols 128t+[0:64]=head a(t), [64:128]=head b(t)
    #   J: (a,b) = (t, t+8) ; I: (a,b) = (t+8, t); v-sec: J pairs (t,t+8), I natural
    wj = wpool.tile([128, 8, 3072], dt.bfloat16)
    wi = wpool.tile([128, 8, 3072], dt.bfloat16)
    wp = wpool.tile([128, 8, 1024], dt.bfloat16)   # rows pair-permuted (t, t+8)
    wf1 = wpool.tile([128, 8, 512], dt.bfloat16)
    wf2 = wpool.tile([128, 4, 1024], dt.bfloat16)
    wconv2 = wpool.tile([128, 120], dt.bfloat16)   # W_Iconv replicated 8x along free

    def load_qkv(dst, src, jlike):
        # src dram [1024, 3072] fp32 (already g-scaled host-side)
        s = src
        for sec in range(2):  # q, k sections: pair-permuted
            for half in range(2):
                # half 0 -> slot [0:64]; half 1 -> slot [64:128] of each pair col
                if jlike:
                    h0 = 0 if half == 0 else 8
                else:
                    h0 = 8 if half == 0 else 0
                # dest cols sec*1024 + 128*t + 64*half + c ; heads h0+t
                d = dst[:, :, :].rearrange("p a (s t o c) -> p a s t o c", s=3, t=8, o=2, c=64)
                sr = s.rearrange("(a p) (s hh c) -> p a s hh c", p=128, s=3, c=64)
                for a in range(8):
                    nc.gpsimd.dma_start(out=d[:, a, sec, :, half, :],
                                        in_=sr[:, a, sec, h0:h0 + 8, :])
        # v section
        if jlike:
            for half in range(2):
                h0 = 0 if half == 0 else 8
                d = dst[:, :, :].rearrange("p a (s t o c) -> p a s t o c", s=3, t=8, o=2, c=64)
                sr = s.rearrange("(a p) (s hh c) -> p a s hh c", p=128, s=3, c=64)
                for a in range(8):
                    nc.gpsimd.dma_start(out=d[:, a, 2, :, half, :],
                                        in_=sr[:, a, 2, h0:h0 + 8, :])
        else:
            dsl = dst[:, :, 2048:3072]
            ssl = s.rearrange("(a p) n -> p a n", p=128)[:, :, 2048:3072]
            nc.gpsimd.dma_start(out=dsl, in_=ssl)

    load_qkv(wj, io["w_jqkv"], True)
    load_qkv(wi, io["w_iqk"], False)

    # W_proj rows pair-permuted: partitions 0:64 <- heads 0..7, 64:128 <- heads 8..15
    # wp[p, a, n]: p<64 -> W_proj[64*a + p, n]; p>=64 -> W_proj[512 + 64*a + (p-64), n]
    nc.gpsimd.dma_start(out=wp[0:64, :, :],
                        in_=io["w_proj"][0:512, :].rearrange("(a p) n -> p a n", p=64))
    nc.gpsimd.dma_start(out=wp[64:128, :, :],
                        in_=io["w_proj"][512:1024, :].rearrange("(a p) n -> p a n", p=64))

    nc.gpsimd.dma_start(out=wf1[:], in_=io["w_fc1"].rearrange("(a p) n -> p a n", p=128))
    nc.gpsimd.dma_start(out=wf2[:], in_=io["w_fc2"].rearrange("(a p) n -> p a n", p=128))
    for g in range(8):
        nc.gpsimd.dma_start(out=wconv2[0:64, 15 * g:15 * g + 15], in_=io["W_Iconv"])
        nc.gpsimd.dma_start(out=wconv2[64:128, 15 * g:15 * g + 15], in_=io["W_Iconv"])

    # ---------------- consts ----------------
    maskt = cpool.tile([120, 120], dt.bfloat16)
    nc.sync.dma_start(maskt[:], io["mask_c"])
    bj_c = cpool.tile([128, 16], dt.float32)
    nc.sync.dma_start(bj_c[:], io["bj_c"])
    bi_c = cpool.tile([128, 24], dt.float32)
    nc.sync.dma_start(bi_c[:], io["bi_c"])
    bvb = cpool.tile([120, 1024], dt.bfloat16)
    nc.gpsimd.dma_start(bvb[:], io["bvb_c"])
    bprojb = cpool.tile([120, 1024], dt.bfloat16)
    nc.gpsimd.dma_start(bprojb[:], io["bprojb_c"])
    bfc2b = cpool.tile([120, 1024], dt.bfloat16)
    nc.gpsimd.dma_start(bfc2b[:], io["bfc2b_c"])
    bf1t = cpool.tile([128, 4], dt.float32)
    nc.sync.dma_start(bf1t[:], io["bfc1t_c"])
    onesc = cpool.tile([120, 1], dt.bfloat16)
    nc.gpsimd.memset(onesc[:], 1.0)
    epst = cpool.tile([128, 1], dt.float32)
    nc.gpsimd.memset(epst[:], EPS)

    # persistent zero-padded LN output slots (rows 120..127 stay zero)
    nslots = []
    for i in range(4):
        t_ = cpool.tile([128, 1024], dt.bfloat16, tag=f"nslot{i}")
        nc.gpsimd.memset(t_[:], 0.0)
        nslots.append(t_)

    jf2 = jf.rearrange("b n c -> (b n) c")
    rf2 = rf.rearrange("b n c -> (b n) c")
    out2 = out.rearrange("b n c -> (b n) c")

    def layernorm_to(dst_slot, src_tile):
        st = smp.tile([120, 2, 6], dt.float32, tag="lnst")
        nc.vector.bn_stats(st[:, 0, :], src_tile[:, 0:512])
        nc.vector.bn_stats(st[:, 1, :], src_tile[:, 512:1024])
        mv = smp.tile([120, 2], dt.float32, tag="lnmv")
        nc.vector.bn_aggr(mv[:], st[:])
        sd = smp.tile([120, 1], dt.float32, tag="lnsd")
        nc.scalar.activation(sd[:], mv[:, 1:2], AF.Sqrt, bias=epst[0:120, :])
        rstd = smp.tile([120, 1], dt.float32, tag="lnrs")
        nc.vector.reciprocal_approx_fast(rstd[:], sd[:])
        nmr = smp.tile([120, 1], dt.float32, tag="lnnm")
        nc.vector.tensor_scalar(nmr[:], mv[:, 0:1], rstd[:], -1.0, ALU.mult, ALU.mult)
        nc.scalar.activation(dst_slot[0:120, :], src_tile[:], AF.Identity,
                             bias=nmr[:], scale=rstd[:])

    # ---------------- main chunk loop ----------------
    for ck in range(nchunk):
        r0 = ck * T

        jt = tokp.tile([120, 1024], dt.float32, tag="jt")
        nc.sync.dma_start(jt[:], jf2[r0:r0 + T, :])
        rt = tokp1.tile([120, 1024], dt.float32, tag="rt_ot")
        nc.sync.dma_start(rt[:], rf2[r0:r0 + T, :])

        xjn = nslots[(ck % 2) * 2 + 0]
        xin = nslots[(ck % 2) * 2 + 1]
        layernorm_to(xjn, jt)
        layernorm_to(xin, rt)

        xjT = fmp.tile([128, 8, 128], dt.bfloat16, tag="xjT")
        nc.sync.dma_start(xjT[:], xjn[:], transpose=True)
        xiT = fmp.tile([128, 8, 128], dt.bfloat16, tag="xiT")
        nc.sync.dma_start(xiT[:], xin[:], transpose=True)

        # ---- QKV GEMMs -> stacks ----
        Qs = fmp1.tile([128, 16, 120], dt.bfloat16, tag="Qs")
        Ks = fmp1.tile([128, 16, 120], dt.bfloat16, tag="Ks")
        IvT = fmp1.tile([128, 8, 128], dt.bfloat16, tag="IvT")

        for sec in range(2):           # 0=q, 1=k
            dst = Qs if sec == 0 else Ks
            for t in range(8):
                ps = gemm_ps.tile([128, 128], dt.float32, tag="gps")
                for c in range(8):
                    nc.tensor.matmul(ps[:], wj[:, c, sec * 1024 + 128 * t:sec * 1024 + 128 * t + 128],
                                     xjT[:, c, :], start=(c == 0), stop=(c == 7))
                # J pairs (t, t+8): top -> stack t rows 0:64, bottom -> stack t+8 rows 64:128
                nc.scalar.activation(dst[0:64, t, :], ps[0:64, 0:120], AF.Identity,
                                     bias=bj_c[0:64, sec * 8 + t:sec * 8 + t + 1])
                nc.scalar.activation(dst[64:128, t + 8, :], ps[64:128, 0:120], AF.Identity,
                                     bias=bj_c[64:128, sec * 8 + t:sec * 8 + t + 1])
            for t in range(8):
                ps = gemm_ps.tile([128, 128], dt.float32, tag="gps")
                for c in range(8):
                    nc.tensor.matmul(ps[:], wi[:, c, sec * 1024 + 128 * t:sec * 1024 + 128 * t + 128],
                                     xiT[:, c, :], start=(c == 0), stop=(c == 7))
                # I pairs (t+8, t): top -> stack t+8 rows 0:64, bottom -> stack t rows 64:128
                nc.scalar.activation(dst[0:64, t + 8, :], ps[0:64, 0:120], AF.Identity,
                                     bias=bi_c[0:64, sec * 8 + t:sec * 8 + t + 1])
                nc.scalar.activation(dst[64:128, t, :], ps[64:128, 0:120], AF.Identity,
                                     bias=bi_c[64:128, sec * 8 + t:sec * 8 + t + 1])

        for t in range(8):             # Iv feature-major (natural head pairs)
            ps = gemm_ps.tile([128, 128], dt.float32, tag="gps")
            for c in range(8):
                nc.tensor.matmul(ps[:], wi[:, c, 2048 + 128 * t:2048 + 128 * t + 128],
                                 xiT[:, c, :], start=(c == 0), stop=(c == 7))
            nc.scalar.activation(IvT[:, t, :], ps[:], AF.Identity,
                                 bias=bi_c[:, 16 + t:16 + t + 1])

        # ---- V token-major (option-B), pair-permuted head cols ----
        vtok = fmp1.tile([128, 1024], dt.bfloat16, tag="vtok")
        for half in range(2):
            ps = gemm_ps.tile([120, 512], dt.float32, tag="gps")
            for c in range(8):
                nc.tensor.matmul(ps[:], xjT[:, c, 0:120],
                                 wj[:, c, 2048 + 512 * half:2048 + 512 * half + 512],
                                 start=(c == 0), stop=(c == 7))
            nc.vector.scalar_tensor_tensor(
                vtok[0:120, 512 * half:512 * half + 512], ps[:], 0.0,
                bvb[:, 512 * half:512 * half + 512], ALU.bypass, ALU.add)

        # ---- attention ----
        if dbg is not None and ck == 0:
            dbg_rs_sb = cpool.tile([1, 120], dt.float32, tag="dbgrs")
            dbg_pv_sb = cpool.tile([128, 120], dt.float32, tag="dbgpv")
        xTx = fmp1.tile([128, 8, 128], dt.bfloat16, tag="xTx")
        for hp in range(8):
            ems = []
            rrs = []
            rcat = smp.tile([1, 240], dt.float32, tag="rcat")
            for hi, h in enumerate((hp, hp + 8)):
                sc = attn_ps.tile([120, 120], dt.float32, tag="aps")
                nc.tensor.matmul(sc[:], Ks[:, h, :], Qs[:, h, :],
                                 start=True, stop=False)
                base = (h % 2) * 64
                nc.tensor.matmul(sc[:], wconv2[base:base + 64, :],
                                 IvT[base:base + 64, h // 2, 0:120],
                                 start=False, stop=True)
                ef = attn_sb.tile([120, 120], dt.bfloat16, tag="ef")
                nc.scalar.activation(ef[:], sc[:], AF.Exp, scale=SCALE)
                em = attn_sb.tile([120, 120], dt.bfloat16, tag="em")
                nc.vector.tensor_mul(em[:], ef[:], maskt[:])
                if dbg is not None and ck == 0 and h == 0:
                    dbg_em0 = em
                rs = attn_ps.tile([1, 120], dt.float32, tag="aps")
                nc.tensor.matmul(rs[:], onesc[:], em[:], start=True, stop=True)
                nc.vector.reciprocal_approx_fast(rcat[:, 120 * hi:120 * hi + 120], rs[:])
                if dbg is not None and ck == 0 and h == 0:
                    nc.vector.tensor_copy(dbg_rs_sb[:], rs[:])
                ems.append(em)
            pv = pv_ps.tile([128, 120], dt.float32, tag="pps")
            nc.tensor.matmul(pv[0:64, :], vtok[0:120, 128 * hp:128 * hp + 64],
                             ems[0][:], start=True, stop=True, tile_position=(0, 0))
            nc.tensor.matmul(pv[64:128, :], vtok[0:120, 128 * hp + 64:128 * hp + 128],
                             ems[1][:], start=True, stop=True, tile_position=(0, 64))
            rb = smp.tile([128, 240], dt.float32, tag="rb")
            nc.gpsimd.partition_broadcast(rb[:], rcat[:])
            nc.vector.tensor_mul(xTx[0:64, hp, 0:120], pv[0:64, :], rb[0:64, 0:120])
            nc.vector.tensor_mul(xTx[64:128, hp, 0:120], pv[64:128, :], rb[64:128, 120:240])
            if dbg is not None and ck == 0 and hp == 0:
                nc.sync.dma_start(dbg["rb0"], rb[:, 0:120])
                nc.vector.tensor_copy(dbg_pv_sb[:], pv[:])

        # ---- proj (option-B) + residual ----
        nc.vector.tensor_add(jt[:], jt[:], bprojb[:])
        jt2 = tokp.tile([120, 1024], dt.float32, tag="jt2")
        for half in range(2):
            ps = gemm_ps.tile([120, 512], dt.float32, tag="gps")
            for c in range(8):
                nc.tensor.matmul(ps[:], xTx[:, c, 0:120],
                                 wp[:, c, 512 * half:512 * half + 512],
                                 start=(c == 0), stop=(c == 7))
            nc.vector.scalar_tensor_tensor(
                jt2[:, 512 * half:512 * half + 512], ps[:], 0.0,
                jt[:, 512 * half:512 * half + 512], ALU.bypass, ALU.add)

        # ---- MLP ----
        xm = nslots[(ck % 2) * 2 + 0]
        layernorm_to(xm, jt2)
        xmT = fmp1.tile([128, 8, 128], dt.bfloat16, tag="xmT")
        nc.sync.dma_start(xmT[:], xm[:], transpose=True)

        h1T = fmp1.tile([128, 4, 128], dt.bfloat16, tag="h1T")
        for t in range(4):
            ps = gemm_ps.tile([128, 128], dt.float32, tag="gps")
            for c in range(8):
                nc.tensor.matmul(ps[:], wf1[:, c, 128 * t:128 * t + 128],
                                 xmT[:, c, :], start=(c == 0), stop=(c == 7))
            nc.scalar.activation(h1T[:, t, :], ps[:], getattr(AF, MLP_ACT),
                                 bias=bf1t[:, t:t + 1])

        nc.vector.tensor_add(jt2[:], jt2[:], bfc2b[:])
        ot = tokp1.tile([120, 1024], dt.float32, tag="rt_ot")
        for half in range(2):
            ps = gemm_ps.tile([120, 512], dt.float32, tag="gps")
            for c in range(4):
                nc.tensor.matmul(ps[:], h1T[:, c, 0:120],
                                 wf2[:, c, 512 * half:512 * half + 512],
                                 start=(c == 0), stop=(c == 3))
            nc.vector.scalar_tensor_tensor(
                ot[:, 512 * half:512 * half + 512], ps[:], 0.0,
                jt2[:, 512 * half:512 * half + 512], ALU.bypass, ALU.add)

        nc.sync.dma_start(out2[r0:r0 + T, :], ot[:])
        if dbg is not None and ck == 0:
            for nm, tl in [("xjn", xjn), ("xin", xin)]:
                nc.sync.dma_start(dbg[nm], tl[:])
            nc.sync.dma_start(dbg["xjT"], xjT[:])
            nc.sync.dma_start(dbg["Qs"], Qs[:])
            nc.sync.dma_start(dbg["Ks"], Ks[:])
            nc.sync.dma_start(dbg["IvT"], IvT[:])
            nc.sync.dma_start(dbg["vtok"], vtok[:])
            nc.sync.dma_start(dbg["xTx"], xTx[:])
            nc.sync.dma_start(dbg["jt2"], jt2[:])
            nc.sync.dma_start(dbg["h1T"], h1T[:])
            nc.sync.dma_start(dbg["em0"], dbg_em0[:])
            nc.sync.dma_start(dbg["xm"], xm[:])
            nc.sync.dma_start(dbg["rs0"], dbg_rs_sb[:])
            nc.sync.dma_start(dbg["pv0"], dbg_pv_sb[:])


def make_consts(W_Jqkv, W_Iqk, W_proj, b_proj, g1, b1, g2, b2, g3, b3,
                W_fc1, b_fc1, W_fc2, b_fc2):
    """Host-side preprocessing: fold LN gamma into weights, compute bias rows."""
    bf16 = ml_dtypes.bfloat16
    wj = (g1[:, None] * W_Jqkv).astype(np.float32)
    wi = (g2[:, None] * W_Iqk).astype(np.float32)
    wf1 = (g3[:, None] * W_fc1).astype(np.float32)
    bjrow = (b1 @ W_Jqkv).astype(np.float32)      # [3072]
    birow = (b2 @ W_Iqk).astype(np.float32)       # [3072]
    bw1row = (b3 @ W_fc1).astype(np.float32)      # [512]

    # mask: block-diag ones [120,120]
    m = np.zeros((120, 120), np.float32)
    for i in range(8):
        m[15 * i:15 * i + 15, 15 * i:15 * i + 15] = 1.0
    mask_c = m.astype(bf16)

    # bj_c [128, 16]: J stacks, pairs (t, t+8): col sec*8+t
    bj = np.zeros((128, 16), np.float32)
    bi = np.zeros((128, 24), np.float32)
    for sec in range(2):
        for t in range(8):
            bj[0:64, sec * 8 + t] = bjrow[sec * 1024 + 64 * t: sec * 1024 + 64 * t + 64]
            bj[64:128, sec * 8 + t] = bjrow[sec * 1024 + 64 * (t + 8): sec * 1024 + 64 * (t + 8) + 64]
            bi[0:64, sec * 8 + t] = birow[sec * 1024 + 64 * (t + 8): sec * 1024 + 64 * (t + 8) + 64]
            bi[64:128, sec * 8 + t] = birow[sec * 1024 + 64 * t: sec * 1024 + 64 * t + 64]
    for t in range(8):
        bi[:, 16 + t] = birow[2048 + 128 * t: 2048 + 128 * t + 128]

    # V token-major bias: cols pair-permuted (t, t+8)
    bvrow = bjrow[2048:3072]
    bvperm = np.zeros(1024, np.float32)
    for t in range(8):
        bvperm[128 * t:128 * t + 64] = bvrow[64 * t:64 * t + 64]
        bvperm[128 * t + 64:128 * t + 128] = bvrow[64 * (t + 8):64 * (t + 8) + 64]
    bvb_c = np.tile(bvperm[None, :], (120, 1)).astype(np.float32)

    bprojb_c = np.tile(b_proj[None, :], (120, 1)).astype(np.float32)
    bfc2b_c = np.tile(b_fc2[None, :], (120, 1)).astype(np.float32)
    bfc1t_c = (b_fc1 + bw1row).reshape(4, 128).T.astype(np.float32).copy()
    return dict(w_jqkv=wj, w_iqk=wi, w_fc1=wf1, mask_c=mask_c, bj_c=bj, bi_c=bi,
                bvb_c=bvb_c, bprojb_c=bprojb_c, bfc2b_c=bfc2b_c, bfc1t_c=bfc1t_c)


def build_nc(nchunk=BC // OB, bc=BC, debug=False):
    import concourse.bacc as bacc
    import concourse.tile as tile
    import concourse.mybir as mybir
    from contextlib import ExitStack

    dt = mybir.dt
    nc = bacc.Bacc("TRN2", target_bir_lowering=False, debug=False,
                   num_devices=NCORES)
    io = {}
    io["joint_feature"] = nc.dram_tensor("joint_feature", [bc, N, C], dt.float32,
                                         kind="ExternalInput").ap()
    io["relation_feature"] = nc.dram_tensor("relation_feature", [bc, N, C], dt.float32,
                                            kind="ExternalInput").ap()
    io["w_jqkv"] = nc.dram_tensor("w_jqkv", [C, 3 * C], dt.float32, kind="ExternalInput").ap()
    io["w_iqk"] = nc.dram_tensor("w_iqk", [C, 3 * C], dt.float32, kind="ExternalInput").ap()
    io["W_Iconv"] = nc.dram_tensor("W_Iconv", [HS, 15], dt.float32, kind="ExternalInput").ap()
    io["w_proj"] = nc.dram_tensor("w_proj", [C, C], dt.float32, kind="ExternalInput").ap()
    io["w_fc1"] = nc.dram_tensor("w_fc1", [C, C // 2], dt.float32, kind="ExternalInput").ap()
    io["w_fc2"] = nc.dram_tensor("w_fc2", [C // 2, C], dt.float32, kind="ExternalInput").ap()
    io["mask_c"] = nc.dram_tensor("mask_c", [120, 120], dt.bfloat16, kind="ExternalInput").ap()
    io["bj_c"] = nc.dram_tensor("bj_c", [128, 16], dt.float32, kind="ExternalInput").ap()
    io["bi_c"] = nc.dram_tensor("bi_c", [128, 24], dt.float32, kind="ExternalInput").ap()
    io["bvb_c"] = nc.dram_tensor("bvb_c", [120, 1024], dt.float32, kind="ExternalInput").ap()
    io["bprojb_c"] = nc.dram_tensor("bprojb_c", [120, 1024], dt.float32, kind="ExternalInput").ap()
    io["bfc2b_c"] = nc.dram_tensor("bfc2b_c", [120, 1024], dt.float32, kind="ExternalInput").ap()
    io["bfc1t_c"] = nc.dram_tensor("bfc1t_c", [128, 4], dt.float32, kind="ExternalInput").ap()
    io["out"] = nc.dram_tensor("out", [bc, N, C], dt.float32, kind="ExternalOutput").ap()
    dbg = None
    if debug:
        dbg = {}
        for nm, shp, dt_ in [("xjn", [128, 1024], dt.bfloat16), ("xin", [128, 1024], dt.bfloat16),
                             ("xjT", [128, 8, 128], dt.bfloat16), ("Qs", [128, 16, 120], dt.bfloat16),
                             ("Ks", [128, 16, 120], dt.bfloat16), ("IvT", [128, 8, 128], dt.bfloat16),
                             ("vtok", [128, 1024], dt.bfloat16), ("xTx", [128, 8, 128], dt.bfloat16),
                             ("jt2", [120, 1024], dt.float32), ("h1T", [128, 4, 128], dt.bfloat16),
                             ("em0", [120, 120], dt.bfloat16), ("xm", [128, 1024], dt.bfloat16),
                             ("rs0", [1, 120], dt.float32),
                             ("rb0", [128, 120], dt.float32), ("pv0", [128, 120], dt.float32)]:
            dbg[nm] = nc.dram_tensor("dbg_" + nm, shp, dt_, kind="ExternalOutput").ap()

    with tile.TileContext(nc) as tc:
        with ExitStack() as ctx:
            tc.ctx = ctx
            _build(tc, io, nchunk, dbg=dbg)
    nc.compile()
    return nc


def kernel(joint_feature, relation_feature, W_Jqkv, W_Iqk, W_Iconv, W_proj, b_proj,
           g_attn1, b_attn1, g_attn2, b_attn2, g_joint, b_joint,
           W_fc1, b_fc1, W_fc2, b_fc2):
    from concourse.bass_utils import run_bass_kernel_spmd

    consts = make_consts(np.asarray(W_Jqkv), np.asarray(W_Iqk), np.asarray(W_proj),
                         np.asarray(b_proj), np.asarray(g_attn1), np.asarray(b_attn1),
                         np.asarray(g_attn2), np.asarray(b_attn2), np.asarray(g_joint),
                         np.asarray(b_joint), np.asarray(W_fc1), np.asarray(b_fc1),
                         np.asarray(W_fc2), np.asarray(b_fc2))
    nc = build_nc()
    jf = np.ascontiguousarray(np.asarray(joint_feature, dtype=np.float32))
    rf = np.ascontiguousarray(np.asarray(relation_feature, dtype=np.float32))
    shared = dict(consts)
    shared["W_Iconv"] = np.asarray(W_Iconv, dtype=np.float32)
    shared["w_proj"] = np.asarray(W_proj, dtype=np.float32)
    shared["w_fc2"] = np.asarray(W_fc2, dtype=np.float32)
    in_maps = []
    for c in range(NCORES):
        m = dict(shared)
        m["joint_feature"] = jf[c * BC:(c + 1) * BC]
        m["relation_feature"] = rf[c * BC:(c + 1) * BC]
        in_maps.append(m)
    res = run_bass_kernel_spmd(nc, in_maps, list(range(NCORES)))
    outs = [res.results[c]["out"] for c in range(NCORES)]
    return np.concatenate(outs, axis=0).astype(np.float32)

